# revision 21
# baseline (speedup 1.0000x reference)
"""MultiHeadedAttention Trainium2 kernel (v3 — exp-stream-centred pipeline).

Problem: B=2, T=2048, D=1024, H=16 heads (DK=64), fp32 in/out, padding mask
on keys. out = softmax(mask(QWq (KWk)^T / 8)) @ (VWv) @ Wo^T + biases.

Sharding (8 cores): core c -> batch b = c//4, head group g = c%4 (4 heads,
256 projection columns). Each core computes its heads' attention and a
partial output projection; host sums the 4 partials per batch (+ bo).

The ScalarE exp stream (128 x ~1.0us ACTIVATEs) is the hard floor (~128us).
Everything else is arranged around keeping it saturated:
  - Phase A: ALL FOUR (m, th) k-projection accumulators run interleaved,
    k-outer, paced by the xk chunk DMAs (PE would otherwise idle on DMA);
    then the same for q. Evacuation copies that gate the first scores go to
    the (still idle) ScalarE; the rest to VectorE. First exp fires ~2us
    after the last xq chunk lands.
  - xv/wv are fp8(e4m3): halves their DMA and SBUF footprint; the V
    projection error (~0.3%) is far inside the 2e-2 budget.
  - vproj rides unit 0 (chunk kc-2 at step kc); V(u0) runs at lag 8 and
    spills its tail into unit 1, which catches up with 2-per-step V; units
    2/3 run the steady lag-2 schedule so the tail V is only 2 chunks.
  - PSUM is exactly 4 rotating 2-bank slots: 2 score tiles in flight + the
    2 V accumulators (o2).  Riders that need a 5th slot stall the exp
    stream, so outproj runs ONLY in the tail (se slots are free there).
  - V accumulators are evacuated to SBUF immediately after stop (frees the
    PSUM slots); normalize runs SBUF-side (recip + gpsimd bcast + bf16 mul),
    with the two head chains interleaved.
  - tail: V(u3) x2 -> outproj(qh=0) (keeps PE warm while norm(u3) runs on
    DVE/GpSimd) -> outproj(qh=1).
"""

import numpy as np
import ml_dtypes

import concourse.bass as bass
import concourse.bacc as bacc
import concourse.tile as tile
from concourse import mybir
from concourse.bass_utils import run_bass_kernel_spmd

B, T, D, H = 2, 2048, 1024, 16
DK = D // H  # 64
GH = 4       # heads per core
GC = GH * DK  # 256 proj columns per core
NCORES = 8
KC = T // 128   # 16 key chunks
DCH = D // 128  # 8 contraction chunks
F32 = mybir.dt.float32
BF16 = mybir.dt.bfloat16
FP8 = mybir.dt.float8e4

MASK_NEG = -30000.0


def build_program(with_bv: bool, with_qkb: bool):
    nc = bacc.Bacc("TRN2")

    # ---- DRAM parameters (per-core shapes) ----
    xq_d = nc.declare_dram_parameter("xq", [DCH, 128, T], BF16, isOutput=False)
    xk_d = nc.declare_dram_parameter("xk", [DCH, 128, T], BF16, isOutput=False)
    xv_d = nc.declare_dram_parameter("xv", [KC, 128, DCH, 128], BF16, isOutput=False)
    wq_d = nc.declare_dram_parameter("wq", [128, DCH, GC], BF16, isOutput=False)
    wk_d = nc.declare_dram_parameter("wk", [128, DCH, GC], BF16, isOutput=False)
    wv_d = nc.declare_dram_parameter("wv", [128, DCH, GC], BF16, isOutput=False)
    wo_d = nc.declare_dram_parameter("wo", [128, 2, D], BF16, isOutput=False)
    mask_d = nc.declare_dram_parameter("maskb", [128, KC], F32, isOutput=False)
    bq_d = nc.declare_dram_parameter("bq", [128, 2], F32, isOutput=False)
    bk_d = nc.declare_dram_parameter("bk", [128, 2], F32, isOutput=False)
    bv_d = nc.declare_dram_parameter("bv", [64, GH], F32, isOutput=False)
    out_d = nc.declare_dram_parameter("out", [T, D], BF16, isOutput=True)

    with tile.TileContext(nc) as tc:
        with (
            tc.tile_pool(name="persist", bufs=1) as pp,
            tc.tile_pool(name="psum", bufs=4, space="PSUM") as psp,
        ):
            # persistent sbuf tensors
            wv_sb = pp.tile([128, DCH, GC], BF16, tag="wv")
            wo_sb = pp.tile([128, 2, D], BF16, tag="wo")
            mask_sb = pp.tile([128, KC], F32, tag="mask")
            bq_sb = pp.tile([128, 2], F32, tag="bq")
            bk_sb = pp.tile([128, 2], F32, tag="bk")
            bv_sb = pp.tile([64, GH], F32, tag="bv")
            qT_sb = pp.tile([128, 2, T], BF16, tag="qT")
            kT_sb = pp.tile([128, 2, T], BF16, tag="kT")
            v_sb = pp.tile([128, KC, GH, 66], BF16, tag="v")
            xh_sb = [pp.tile([128, 2, 1024], BF16, tag=f"xh{q}", name=f"xh{q}")
                     for q in (0, 1)]
            # prime the ScalarE activation table (exp) before any real work
            scrap_i = pp.tile([1, 16], F32, tag="scrap_i")
            scrap_o = pp.tile([1, 16], BF16, tag="scrap_o")
            nc.vector.memset(scrap_i[:], 0.0)
            nc.scalar.activation(scrap_o[:], scrap_i[:],
                                 mybir.ActivationFunctionType.Exp)
            nc.vector.memset(v_sb[:, :, :, 64:65], 1.0)

            xvp_cm = tc.tile_pool(name="xv", bufs=1)
            xvp = xvp_cm.__enter__()
            xv_sb = [xvp.tile([128, DCH, 128], BF16, tag="xv", bufs=8,
                              name=f"xv{t}") for t in range(KC)]

            bc_pools = (
                tc.tile_pool(name="expp", bufs=26),
                tc.tile_pool(name="outp", bufs=4),
                tc.tile_pool(name="normp", bufs=2),
            )
            exp_pool = bc_pools[0].__enter__()
            out_pool = bc_pools[1].__enter__()
            norm_pool = bc_pools[2].__enter__()

            xqp_cm = tc.tile_pool(name="xqp", bufs=1)
            xqp = xqp_cm.__enter__()
            wq_sb = xqp.tile([128, DCH, GC], BF16, tag="wq", name="wq_sb")
            xq_sb = [xqp.tile([128, 1024], BF16, tag="xq", bufs=4, name=f"xq{i}")
                     for i in range(2 * DCH)]
            xkp_cm = tc.tile_pool(name="xkp", bufs=1)
            xkp = xkp_cm.__enter__()
            wk_sb = xkp.tile([128, DCH, GC], BF16, tag="wk", name="wk_sb")
            xk_sb = [xkp.tile([128, 1024], BF16, tag="xk", bufs=4, name=f"xk{i}")
                     for i in range(2 * DCH)]

            # ---- DMAs: interleaved so each projection wave's data lands
            # just in time: xk-th0, xq-th0 (gates first exp), xk-th1,
            # xv token-chunks, xq-th1, rest.
            nc.sync.dma_start(out=wk_sb[:], in_=wk_d[:])
            nc.sync.dma_start(out=bk_sb[:], in_=bk_d[:])
            for k in range(DCH):
                nc.sync.dma_start(out=xk_sb[k][:], in_=xk_d[k][:, 0:1024])
            for k in range(DCH):
                nc.sync.dma_start(out=xk_sb[DCH + k][:],
                                  in_=xk_d[k][:, 1024:2048])
            nc.sync.dma_start(out=wq_sb[:], in_=wq_d[:])
            nc.sync.dma_start(out=bq_sb[:], in_=bq_d[:])
            nc.sync.dma_start(out=mask_sb[:], in_=mask_d[:])
            for k in range(DCH):
                nc.sync.dma_start(out=xq_sb[k][:], in_=xq_d[k][:, 0:1024])
            for k in range(DCH):
                nc.sync.dma_start(out=xq_sb[DCH + k][:],
                                  in_=xq_d[k][:, 1024:2048])
            nc.sync.dma_start(out=wv_sb[:], in_=wv_d[:])
            for t in range(KC):
                nc.sync.dma_start(out=xv_sb[t][:], in_=xv_d[t])
            nc.sync.dma_start(out=bv_sb[:], in_=bv_d[:])
            nc.sync.dma_start(out=wo_sb[:], in_=wo_d[:])

            def emit_projw(w_sb, x_sb, dst, b_sb, th, engs):
                """One token-half wave of a projection: both m accumulators,
                k-outer so they advance with the x-half-chunk DMAs."""
                pst = {m: psp.tile([128, 1024], F32, tag="ps",
                                   name=f"pst{m}") for m in (0, 1)}
                for k in range(DCH):
                    for m in (0, 1):
                        for n in (0, 1):
                            nc.tensor.matmul(
                                pst[m][:, n * 512:(n + 1) * 512],
                                w_sb[:, k, m * 128:(m + 1) * 128],
                                x_sb[th * DCH + k][:, n * 512:(n + 1) * 512],
                                start=(k == 0), stop=(k == DCH - 1),
                                skip_group_check=True,
                            )
                for m in (0, 1):
                    sl = dst[:, m, th * 1024:(th + 1) * 1024]
                    if with_qkb:
                        nc.vector.tensor_scalar_add(sl, pst[m][:],
                                                    b_sb[:, m:m + 1])
                    elif engs[m] == "s":
                        nc.scalar.copy(sl, pst[m][:])
                    else:
                        nc.vector.tensor_copy(sl, pst[m][:])

            def emit_projq(w_sb, x_sb, dst, b_sb, th, m, eng):
                """One (m, th) quarter of a projection — 1 PSUM slot."""
                pst = psp.tile([128, 1024], F32, tag="ps", name=f"pq{m}{th}")
                for k in range(DCH):
                    for n in (0, 1):
                        nc.tensor.matmul(
                            pst[:, n * 512:(n + 1) * 512],
                            w_sb[:, k, m * 128:(m + 1) * 128],
                            x_sb[th * DCH + k][:, n * 512:(n + 1) * 512],
                            start=(k == 0), stop=(k == DCH - 1),
                            skip_group_check=True,
                        )
                sl = dst[:, m, th * 1024:(th + 1) * 1024]
                if with_qkb:
                    nc.vector.tensor_scalar_add(sl, pst[:], b_sb[:, m:m + 1])
                elif eng == "s":
                    nc.scalar.copy(sl, pst[:])
                else:
                    nc.vector.tensor_copy(sl, pst[:])

            def emit_vproj(tcn):
                ps = psp.tile([128, GH, 64], F32, tag="ps", name="vps")
                for k in range(DCH):
                    nc.tensor.matmul(
                        ps[:],
                        xv_sb[tcn][:, k, :],
                        wv_sb[:, k, :],
                        start=(k == 0), stop=(k == DCH - 1),
                        skip_group_check=True,
                    )
                nc.vector.tensor_copy(v_sb[:, tcn, :, 0:64], ps[:])

            # ---- Phase A: full k projection (xk-paced), then qproj-th0
            # (gates the first scores).  qproj-th1 rides unit 0 as two
            # 1-slot quarter-waves.  ScalarE takes pre-exp-stream copies.
            emit_projw(wk_sb, xk_sb, kT_sb, bk_sb, 0, ("s", "s"))
            emit_projw(wk_sb, xk_sb, kT_sb, bk_sb, 1, ("s", "v"))
            xkp_cm.__exit__(None, None, None)
            emit_projw(wq_sb, xq_sb, qT_sb, bq_sb, 0, ("s", "v"))

            def emit_scores(qh, pr, kc, exs):
                q0 = qh * 1024
                se = [psp.tile([128, 1024], F32, tag="ps", name="se")
                      for _ in range(2)]
                for n in range(2):
                    for hh in range(2):
                        pb = 64 * hh
                        nc.tensor.matmul(
                            se[hh][:, n * 512:(n + 1) * 512],
                            kT_sb[pb:pb + 64, pr, kc * 128:(kc + 1) * 128],
                            qT_sb[pb:pb + 64, pr,
                                  q0 + n * 512:q0 + (n + 1) * 512],
                            start=True, stop=True,
                        )
                ex = [exp_pool.tile([128, 1024], BF16, tag="ex", name="ex")
                      for _ in range(2)]
                for hh in range(2):
                    nc.scalar.activation(
                        ex[hh][:], se[hh][:],
                        mybir.ActivationFunctionType.Exp,
                        bias=mask_sb[:, kc:kc + 1],
                        scale=float(DK) ** -0.5,
                    )
                exs.append(ex)

            def emit_v(pr, kc, o2, exs):
                for hh in range(2):
                    h = 2 * pr + hh
                    for n in range(2):
                        nc.tensor.matmul(
                            o2[hh][:, n * 512:(n + 1) * 512],
                            v_sb[:, kc, h, 0:65],
                            exs[kc][hh][:, n * 512:(n + 1) * 512],
                            start=(kc == 0), stop=(kc == KC - 1),
                            skip_group_check=True,
                        )

            def emit_norm(qh, pr, o2):
                """Evacuate o2 fast (frees PSUM), then normalize SBUF-side.
                The two head chains are interleaved so the gpsimd broadcasts
                overlap the DVE work."""
                rr, xr, rc, rrb, rb = [None, None], [None, None], \
                    [None, None], [None, None], [None, None]
                for hh in (1, 0):
                    rr[hh] = norm_pool.tile([1, 1024], F32, tag="rr",
                                            name="rr")
                    nc.vector.tensor_copy(rr[hh][:], o2[hh][64:65, :])
                    xr[hh] = norm_pool.tile([64, 1024], BF16, tag="xr",
                                            name="xr")
                    nc.vector.tensor_copy(xr[hh][:], o2[hh][0:64, :])
                for hh in (1, 0):
                    rc[hh] = norm_pool.tile([1, 1024], F32, tag="rc",
                                            name="rc")
                    nc.vector.reciprocal_approx_fast(rc[hh][:], rr[hh][:])
                    rrb[hh] = norm_pool.tile([1, 1024], BF16, tag="rrb",
                                             bufs=1, name="rrb")
                    nc.vector.tensor_copy(rrb[hh][:], rc[hh][:])
                    rb[hh] = norm_pool.tile([64, 1024], BF16, tag="rb",
                                            name="rb")
                    nc.gpsimd.partition_broadcast(rb[hh][:], rrb[hh][:])
                for hh in (1, 0):
                    if hh == 0:
                        nc.vector.tensor_mul(
                            xh_sb[qh][0:64, pr, :], xr[hh][:], rb[hh][:])
                        if with_bv:
                            nc.vector.tensor_scalar_add(
                                xh_sb[qh][0:64, pr, :],
                                xh_sb[qh][0:64, pr, :],
                                bv_sb[:, 2 * pr:2 * pr + 1])
                    else:
                        tmp = norm_pool.tile([64, 1024], BF16, tag="tmp",
                                             name="tmp")
                        nc.vector.tensor_mul(tmp[:], xr[hh][:], rb[hh][:])
                        if with_bv:
                            nc.vector.tensor_scalar_add(
                                tmp[:], tmp[:],
                                bv_sb[:, 2 * pr + 1:2 * pr + 2])
                        nc.sync.dma_start(
                            out=xh_sb[qh][64:128, pr, :], in_=tmp[:])

            def emit_outproj(qh, tr, ceng="v"):
                tcn = qh * 8 + tr
                po = psp.tile([128, 1024], F32, tag="ps", name="po")
                for m in range(2):
                    for n in range(2):
                        nc.tensor.matmul(
                            po[:, n * 512:(n + 1) * 512],
                            xh_sb[qh][:, m, tr * 128:(tr + 1) * 128],
                            wo_sb[:, m, n * 512:(n + 1) * 512],
                            start=(m == 0), stop=(m == 1),
                            skip_group_check=True,
                        )
                ot = out_pool.tile([128, 1024], BF16, tag="ot")
                if ceng == "s":
                    nc.scalar.copy(ot[:], po[:])
                else:
                    nc.vector.tensor_copy(ot[:], po[:])
                nc.sync.dma_start(
                    out=out_d[tcn * 128:(tcn + 1) * 128, :], in_=ot[:])

            # units ordered (0,0),(1,0),(0,1),(1,1).  Riders per unit:
            #  u0: qproj-th1 quarter-waves (kc2, kc4), vproj(kc-5) from kc5,
            #      V(u0, kc-8) from kc8 (lag 8 while vproj streams in).
            #  u1: vproj tail, V(u0) tail catch-up (2/kc), then V(u1) at
            #      1.5/kc; the last 4 V(u1) spill to u2.
            #  u2/u3: prev-unit V tail at kc0-1, norm(prev) at kc2, own V at
            #      lag 3; 3-chunk V tail spills forward.
            units = [(0, 0), (1, 0), (0, 1), (1, 1)]
            o2_u, exs_u = [], []
            for ui, (qh, pr) in enumerate(units):
                o2 = [psp.tile([65, 1024], F32, tag="ps", name="o2")
                      for _ in range(2)]
                exs = []
                o2_u.append(o2)
                exs_u.append(exs)
                ppr = units[ui - 1][1] if ui else 0
                pqh = units[ui - 1][0] if ui else 0
                for kc in range(KC):
                    emit_scores(qh, pr, kc, exs)
                    if ui == 0:
                        if kc == 5:
                            emit_projq(wq_sb, xq_sb, qT_sb, bq_sb, 1, 0, "v")
                        elif kc == 7:
                            emit_projq(wq_sb, xq_sb, qT_sb, bq_sb, 1, 1, "v")
                        elif kc >= 6:
                            emit_vproj(kc - 7 if kc >= 8 else 0)
                        if kc >= 8:
                            emit_v(0, kc - 8, o2, exs)
                    elif ui == 1:
                        if kc <= 2:
                            emit_vproj(9 + 2 * kc)
                            emit_vproj(10 + 2 * kc)
                        elif kc == 3:
                            emit_vproj(15)
                            emit_v(0, 8, o2_u[0], exs_u[0])
                        elif kc <= 6:
                            emit_v(0, 2 * kc + 1, o2_u[0], exs_u[0])
                            emit_v(0, 2 * kc + 2, o2_u[0], exs_u[0])
                        elif kc == 7:
                            emit_v(0, 15, o2_u[0], exs_u[0])
                            emit_norm(0, 0, o2_u[0])
                        else:
                            # kc 8..15 -> V(u1, 0..11) at 1.5/kc
                            c0 = (3 * (kc - 8)) // 2
                            c1 = (3 * (kc - 7)) // 2
                            for c in range(c0, c1):
                                emit_v(pr, c, o2, exs)
                    else:
                        if kc == 0:
                            for c in (12, 13) if ui == 2 else (13, 14):
                                emit_v(ppr, c, o2_u[ui - 1], exs_u[ui - 1])
                        elif kc == 1:
                            for c in ((14, 15) if ui == 2 else (15,)):
                                emit_v(ppr, c, o2_u[ui - 1], exs_u[ui - 1])
                        elif kc == 2:
                            emit_norm(pqh, ppr, o2_u[ui - 1])
                        if kc >= 3:
                            emit_v(pr, kc - 3, o2, exs)

            # tail: V(u3) x3 -> outproj(0) brackets norm(u3) -> outproj(1)
            for kc in range(KC - 3, KC):
                emit_v(pr, kc, o2, exs)
            for tr in range(4):
                emit_outproj(0, tr, ceng="sv"[tr % 2])
            emit_norm(1, 1, o2)
            for tr in range(4, 8):
                emit_outproj(0, tr, ceng="sv"[tr % 2])
            for tr in range(8):
                emit_outproj(1, tr, ceng="sv"[tr % 2])

            xqp_cm.__exit__(None, None, None)

            for _p in reversed(bc_pools):
                _p.__exit__(None, None, None)
            xvp_cm.__exit__(None, None, None)

    nc.compile()
    return nc


_CACHE = {}


def _get_program(with_bv: bool, with_qkb: bool):
    key = (with_bv, with_qkb)
    if key not in _CACHE:
        _CACHE[key] = build_program(with_bv, with_qkb)
    return _CACHE[key]


def make_in_maps(query, key, value, mask, Wq, bq, Wk, bk, Wv, bv, Wo, bo):
    bf = ml_dtypes.bfloat16
    f8 = ml_dtypes.float8_e4m3
    # transposed activations are shared by the 4 cores of each batch
    xt = {}
    for nm, x, dt in (("xq", query, bf), ("xk", key, bf)):
        for b in range(B):
            xt[nm, b] = np.ascontiguousarray(
                x[b].T.reshape(DCH, 128, T)).astype(dt)
    for b in range(B):
        # [KC, 128, DCH, 128]: token-chunk major, partition = D-row within
        # chunk, 2KB contiguous per-partition lines for full DMA speed
        xt["xv", b] = np.ascontiguousarray(
            value[b].reshape(KC, 128, DCH, 128).transpose(0, 3, 2, 1)
        ).astype(bf)
    in_maps = []
    for c in range(NCORES):
        b, g = c // 4, c % 4
        cols = slice(GC * g, GC * (g + 1))
        m = {}
        for nm in ("xq", "xk", "xv"):
            m[nm] = xt[nm, b]
        for nm, W, dt in (("wq", Wq, bf), ("wk", Wk, bf), ("wv", Wv, bf)):
            m[nm] = np.ascontiguousarray(
                W[cols, :].T.reshape(DCH, 128, GC).transpose(1, 0, 2)
            ).astype(dt)
        m["wo"] = np.ascontiguousarray(
            Wo[:, cols].T.reshape(2, 128, D).transpose(1, 0, 2)).astype(bf)
        mb = np.where(mask[b, 0] != 0, 0.0, MASK_NEG).astype(np.float32)
        m["maskb"] = np.ascontiguousarray(mb.reshape(KC, 128).T)
        m["bq"] = np.ascontiguousarray(
            bq[cols].reshape(2, 128).T.astype(np.float32))
        m["bk"] = np.ascontiguousarray(
            bk[cols].reshape(2, 128).T.astype(np.float32))
        m["bv"] = np.ascontiguousarray(
            bv[cols].reshape(GH, 64).T.astype(np.float32))
        in_maps.append(m)
    return in_maps


def kernel(query, key, value, mask, Wq, bq, Wk, bk, Wv, bv, Wo, bo,
           _trace=False):
    query, key, value = (np.asarray(a, np.float32) for a in (query, key, value))
    mask = np.asarray(mask)
    with_bv = bool(np.any(np.asarray(bv)))
    with_qkb = bool(np.any(np.asarray(bq))) or bool(np.any(np.asarray(bk)))
    nc = _get_program(with_bv, with_qkb)
    in_maps = make_in_maps(query, key, value, mask, Wq, bq, Wk, bk, Wv, bv,
                           Wo, bo)
    res = run_bass_kernel_spmd(nc, in_maps, list(range(NCORES)), trace=_trace)
    out = np.zeros((B, T, D), np.float32)
    for c in range(NCORES):
        out[c // 4] += res.results[c]["out"].astype(np.float32)
    out += np.asarray(bo, np.float32)[None, None, :]
    if _trace:
        kernel.last_exec_time_ns = res.exec_time_ns
        kernel.last_results = res
    return out


# revision 23
# speedup vs baseline: 1.0025x; 1.0025x over previous
"""MultiHeadedAttention Trainium2 kernel (v3 — exp-stream-centred pipeline).

Problem: B=2, T=2048, D=1024, H=16 heads (DK=64), fp32 in/out, padding mask
on keys. out = softmax(mask(QWq (KWk)^T / 8)) @ (VWv) @ Wo^T + biases.

Sharding (8 cores): core c -> batch b = c//4, head group g = c%4 (4 heads,
256 projection columns). Each core computes its heads' attention and a
partial output projection; host sums the 4 partials per batch (+ bo).

The ScalarE exp stream (128 x ~1.0us ACTIVATEs) is the hard floor (~128us).
Everything else is arranged around keeping it saturated:
  - Phase A: ALL FOUR (m, th) k-projection accumulators run interleaved,
    k-outer, paced by the xk chunk DMAs (PE would otherwise idle on DMA);
    then the same for q. Evacuation copies that gate the first scores go to
    the (still idle) ScalarE; the rest to VectorE. First exp fires ~2us
    after the last xq chunk lands.
  - xv/wv are fp8(e4m3): halves their DMA and SBUF footprint; the V
    projection error (~0.3%) is far inside the 2e-2 budget.
  - vproj rides unit 0 (chunk kc-2 at step kc); V(u0) runs at lag 8 and
    spills its tail into unit 1, which catches up with 2-per-step V; units
    2/3 run the steady lag-2 schedule so the tail V is only 2 chunks.
  - PSUM is exactly 4 rotating 2-bank slots: 2 score tiles in flight + the
    2 V accumulators (o2).  Riders that need a 5th slot stall the exp
    stream, so outproj runs ONLY in the tail (se slots are free there).
  - V accumulators are evacuated to SBUF immediately after stop (frees the
    PSUM slots); normalize runs SBUF-side (recip + gpsimd bcast + bf16 mul),
    with the two head chains interleaved.
  - tail: V(u3) x2 -> outproj(qh=0) (keeps PE warm while norm(u3) runs on
    DVE/GpSimd) -> outproj(qh=1).
"""

import numpy as np
import ml_dtypes

import concourse.bass as bass
import concourse.bacc as bacc
import concourse.tile as tile
from concourse import mybir
from concourse.bass_utils import run_bass_kernel_spmd

B, T, D, H = 2, 2048, 1024, 16
DK = D // H  # 64
GH = 4       # heads per core
GC = GH * DK  # 256 proj columns per core
NCORES = 8
KC = T // 128   # 16 key chunks
DCH = D // 128  # 8 contraction chunks
F32 = mybir.dt.float32
BF16 = mybir.dt.bfloat16
FP8 = mybir.dt.float8e4

MASK_NEG = -30000.0


def build_program(with_bv: bool, with_qkb: bool):
    nc = bacc.Bacc("TRN2")

    # ---- DRAM parameters (per-core shapes) ----
    xq_d = nc.declare_dram_parameter("xq", [DCH, 128, T], BF16, isOutput=False)
    xk_d = nc.declare_dram_parameter("xk", [DCH, 128, T], BF16, isOutput=False)
    xv_d = nc.declare_dram_parameter("xv", [KC, 128, DCH, 128], BF16, isOutput=False)
    wq_d = nc.declare_dram_parameter("wq", [128, DCH, GC], BF16, isOutput=False)
    wk_d = nc.declare_dram_parameter("wk", [128, DCH, GC], BF16, isOutput=False)
    wv_d = nc.declare_dram_parameter("wv", [128, DCH, GC], BF16, isOutput=False)
    wo_d = nc.declare_dram_parameter("wo", [128, 2, D], BF16, isOutput=False)
    mask_d = nc.declare_dram_parameter("maskb", [128, KC], F32, isOutput=False)
    bq_d = nc.declare_dram_parameter("bq", [128, 2], F32, isOutput=False)
    bk_d = nc.declare_dram_parameter("bk", [128, 2], F32, isOutput=False)
    bv_d = nc.declare_dram_parameter("bv", [64, GH], F32, isOutput=False)
    out_d = nc.declare_dram_parameter("out", [T, D], BF16, isOutput=True)

    with tile.TileContext(nc) as tc:
        with (
            tc.tile_pool(name="persist", bufs=1) as pp,
            tc.tile_pool(name="psum", bufs=4, space="PSUM") as psp,
        ):
            # persistent sbuf tensors
            wv_sb = pp.tile([128, DCH, GC], BF16, tag="wv")
            wo_sb = pp.tile([128, 2, D], BF16, tag="wo")
            mask_sb = pp.tile([128, KC], F32, tag="mask")
            bq_sb = pp.tile([128, 2], F32, tag="bq")
            bk_sb = pp.tile([128, 2], F32, tag="bk")
            bv_sb = pp.tile([64, GH], F32, tag="bv")
            qT_sb = pp.tile([128, 2, T], BF16, tag="qT")
            kT_sb = pp.tile([128, 2, T], BF16, tag="kT")
            v_sb = pp.tile([128, KC, GH, 66], BF16, tag="v")
            xh_sb = [pp.tile([128, 2, 1024], BF16, tag=f"xh{q}", name=f"xh{q}")
                     for q in (0, 1)]
            # prime the ScalarE activation table (exp) before any real work
            scrap_i = pp.tile([1, 16], F32, tag="scrap_i")
            scrap_o = pp.tile([1, 16], BF16, tag="scrap_o")
            nc.vector.memset(scrap_i[:], 0.0)
            nc.scalar.activation(scrap_o[:], scrap_i[:],
                                 mybir.ActivationFunctionType.Exp)
            nc.vector.memset(v_sb[:, :, :, 64:65], 1.0)

            xvp_cm = tc.tile_pool(name="xv", bufs=1)
            xvp = xvp_cm.__enter__()
            xv_sb = [xvp.tile([128, DCH, 128], BF16, tag="xv", bufs=8,
                              name=f"xv{t}") for t in range(KC)]

            bc_pools = (
                tc.tile_pool(name="expp", bufs=28),
                tc.tile_pool(name="outp", bufs=4),
                tc.tile_pool(name="normp", bufs=2),
            )
            exp_pool = bc_pools[0].__enter__()
            out_pool = bc_pools[1].__enter__()
            norm_pool = bc_pools[2].__enter__()

            xqp_cm = tc.tile_pool(name="xqp", bufs=1)
            xqp = xqp_cm.__enter__()
            wq_sb = xqp.tile([128, DCH, GC], BF16, tag="wq", name="wq_sb")
            xq_sb = [xqp.tile([128, 1024], BF16, tag="xq", bufs=4, name=f"xq{i}")
                     for i in range(2 * DCH)]
            xkp_cm = tc.tile_pool(name="xkp", bufs=1)
            xkp = xkp_cm.__enter__()
            wk_sb = xkp.tile([128, DCH, GC], BF16, tag="wk", name="wk_sb")
            xk_sb = [xkp.tile([128, 1024], BF16, tag="xk", bufs=4, name=f"xk{i}")
                     for i in range(2 * DCH)]

            # ---- DMAs: interleaved so each projection wave's data lands
            # just in time: xk-th0, xq-th0 (gates first exp), xk-th1,
            # xv token-chunks, xq-th1, rest.
            nc.sync.dma_start(out=wk_sb[:], in_=wk_d[:])
            nc.sync.dma_start(out=bk_sb[:], in_=bk_d[:])
            for k in range(DCH):
                nc.sync.dma_start(out=xk_sb[k][:], in_=xk_d[k][:, 0:1024])
            nc.sync.dma_start(out=wq_sb[:], in_=wq_d[:])
            nc.sync.dma_start(out=bq_sb[:], in_=bq_d[:])
            nc.sync.dma_start(out=mask_sb[:], in_=mask_d[:])
            for k in range(DCH):
                nc.sync.dma_start(out=xq_sb[k][:], in_=xq_d[k][:, 0:1024])
            for k in range(DCH):
                nc.sync.dma_start(out=xk_sb[DCH + k][:],
                                  in_=xk_d[k][:, 1024:2048])
            for k in range(DCH):
                nc.sync.dma_start(out=xq_sb[DCH + k][:],
                                  in_=xq_d[k][:, 1024:2048])
            nc.sync.dma_start(out=wv_sb[:], in_=wv_d[:])
            for t in range(KC):
                nc.sync.dma_start(out=xv_sb[t][:], in_=xv_d[t])
            nc.sync.dma_start(out=bv_sb[:], in_=bv_d[:])
            nc.sync.dma_start(out=wo_sb[:], in_=wo_d[:])

            def emit_projw(w_sb, x_sb, dst, b_sb, th, engs):
                """One token-half wave of a projection: both m accumulators,
                k-outer so they advance with the x-half-chunk DMAs."""
                pst = {m: psp.tile([128, 1024], F32, tag="ps",
                                   name=f"pst{m}") for m in (0, 1)}
                for k in range(DCH):
                    for m in (0, 1):
                        for n in (0, 1):
                            nc.tensor.matmul(
                                pst[m][:, n * 512:(n + 1) * 512],
                                w_sb[:, k, m * 128:(m + 1) * 128],
                                x_sb[th * DCH + k][:, n * 512:(n + 1) * 512],
                                start=(k == 0), stop=(k == DCH - 1),
                                skip_group_check=True,
                            )
                for m in (0, 1):
                    sl = dst[:, m, th * 1024:(th + 1) * 1024]
                    if with_qkb:
                        nc.vector.tensor_scalar_add(sl, pst[m][:],
                                                    b_sb[:, m:m + 1])
                    elif engs[m] == "s":
                        nc.scalar.copy(sl, pst[m][:])
                    else:
                        nc.vector.tensor_copy(sl, pst[m][:])

            def emit_projq(w_sb, x_sb, dst, b_sb, th, m, eng):
                """One (m, th) quarter of a projection — 1 PSUM slot."""
                pst = psp.tile([128, 1024], F32, tag="ps", name=f"pq{m}{th}")
                for k in range(DCH):
                    for n in (0, 1):
                        nc.tensor.matmul(
                            pst[:, n * 512:(n + 1) * 512],
                            w_sb[:, k, m * 128:(m + 1) * 128],
                            x_sb[th * DCH + k][:, n * 512:(n + 1) * 512],
                            start=(k == 0), stop=(k == DCH - 1),
                            skip_group_check=True,
                        )
                sl = dst[:, m, th * 1024:(th + 1) * 1024]
                if with_qkb:
                    nc.vector.tensor_scalar_add(sl, pst[:], b_sb[:, m:m + 1])
                elif eng == "s":
                    nc.scalar.copy(sl, pst[:])
                else:
                    nc.vector.tensor_copy(sl, pst[:])

            def emit_vproj(tcn):
                ps = psp.tile([128, GH, 64], F32, tag="ps", name="vps")
                for k in range(DCH):
                    nc.tensor.matmul(
                        ps[:],
                        xv_sb[tcn][:, k, :],
                        wv_sb[:, k, :],
                        start=(k == 0), stop=(k == DCH - 1),
                        skip_group_check=True,
                    )
                nc.vector.tensor_copy(v_sb[:, tcn, :, 0:64], ps[:])

            # ---- Phase A: full k projection (xk-paced), then qproj-th0
            # (gates the first scores).  qproj-th1 rides unit 0 as two
            # 1-slot quarter-waves.  ScalarE takes pre-exp-stream copies.
            emit_projw(wk_sb, xk_sb, kT_sb, bk_sb, 0, ("s", "s"))
            emit_projw(wq_sb, xq_sb, qT_sb, bq_sb, 0, ("s", "v"))

            def emit_scores(qh, pr, kc, exs):
                q0 = qh * 1024
                se = [psp.tile([128, 1024], F32, tag="ps", name="se")
                      for _ in range(2)]
                for n in range(2):
                    for hh in range(2):
                        pb = 64 * hh
                        nc.tensor.matmul(
                            se[hh][:, n * 512:(n + 1) * 512],
                            kT_sb[pb:pb + 64, pr, kc * 128:(kc + 1) * 128],
                            qT_sb[pb:pb + 64, pr,
                                  q0 + n * 512:q0 + (n + 1) * 512],
                            start=True, stop=True,
                        )
                ex = [exp_pool.tile([128, 1024], BF16, tag="ex", name="ex")
                      for _ in range(2)]
                for hh in range(2):
                    nc.scalar.activation(
                        ex[hh][:], se[hh][:],
                        mybir.ActivationFunctionType.Exp,
                        bias=mask_sb[:, kc:kc + 1],
                        scale=float(DK) ** -0.5,
                    )
                exs.append(ex)

            def emit_v(pr, kc, o2, exs):
                for hh in range(2):
                    h = 2 * pr + hh
                    for n in range(2):
                        nc.tensor.matmul(
                            o2[hh][:, n * 512:(n + 1) * 512],
                            v_sb[:, kc, h, 0:65],
                            exs[kc][hh][:, n * 512:(n + 1) * 512],
                            start=(kc == 0), stop=(kc == KC - 1),
                            skip_group_check=True,
                        )

            def emit_norm(qh, pr, o2):
                """Evacuate o2 fast (frees PSUM), then normalize SBUF-side.
                The two head chains are interleaved so the gpsimd broadcasts
                overlap the DVE work."""
                rr, xr, rc, rrb, rb = [None, None], [None, None], \
                    [None, None], [None, None], [None, None]
                for hh in (1, 0):
                    rr[hh] = norm_pool.tile([1, 1024], F32, tag="rr",
                                            name="rr")
                    nc.vector.tensor_copy(rr[hh][:], o2[hh][64:65, :])
                    xr[hh] = norm_pool.tile([64, 1024], BF16, tag="xr",
                                            name="xr")
                    nc.vector.tensor_copy(xr[hh][:], o2[hh][0:64, :])
                for hh in (1, 0):
                    rc[hh] = norm_pool.tile([1, 1024], F32, tag="rc",
                                            name="rc")
                    nc.vector.reciprocal_approx_fast(rc[hh][:], rr[hh][:])
                    rrb[hh] = norm_pool.tile([1, 1024], BF16, tag="rrb",
                                             bufs=1, name="rrb")
                    nc.vector.tensor_copy(rrb[hh][:], rc[hh][:])
                    rb[hh] = norm_pool.tile([64, 1024], BF16, tag="rb",
                                            name="rb")
                    nc.gpsimd.partition_broadcast(rb[hh][:], rrb[hh][:])
                for hh in (1, 0):
                    if hh == 0:
                        nc.vector.tensor_mul(
                            xh_sb[qh][0:64, pr, :], xr[hh][:], rb[hh][:])
                        if with_bv:
                            nc.vector.tensor_scalar_add(
                                xh_sb[qh][0:64, pr, :],
                                xh_sb[qh][0:64, pr, :],
                                bv_sb[:, 2 * pr:2 * pr + 1])
                    else:
                        tmp = norm_pool.tile([64, 1024], BF16, tag="tmp",
                                             name="tmp")
                        nc.vector.tensor_mul(tmp[:], xr[hh][:], rb[hh][:])
                        if with_bv:
                            nc.vector.tensor_scalar_add(
                                tmp[:], tmp[:],
                                bv_sb[:, 2 * pr + 1:2 * pr + 2])
                        nc.sync.dma_start(
                            out=xh_sb[qh][64:128, pr, :], in_=tmp[:])

            def emit_outproj(qh, tr, ceng="v"):
                tcn = qh * 8 + tr
                po = psp.tile([128, 1024], F32, tag="ps", name="po")
                for m in range(2):
                    for n in range(2):
                        nc.tensor.matmul(
                            po[:, n * 512:(n + 1) * 512],
                            xh_sb[qh][:, m, tr * 128:(tr + 1) * 128],
                            wo_sb[:, m, n * 512:(n + 1) * 512],
                            start=(m == 0), stop=(m == 1),
                            skip_group_check=True,
                        )
                ot = out_pool.tile([128, 1024], BF16, tag="ot")
                if ceng == "s":
                    nc.scalar.copy(ot[:], po[:])
                else:
                    nc.vector.tensor_copy(ot[:], po[:])
                nc.sync.dma_start(
                    out=out_d[tcn * 128:(tcn + 1) * 128, :], in_=ot[:])

            # units ordered (0,0),(1,0),(0,1),(1,1).  Riders per unit:
            #  u0: qproj-th1 quarter-waves (kc2, kc4), vproj(kc-5) from kc5,
            #      V(u0, kc-8) from kc8 (lag 8 while vproj streams in).
            #  u1: vproj tail, V(u0) tail catch-up (2/kc), then V(u1) at
            #      1.5/kc; the last 4 V(u1) spill to u2.
            #  u2/u3: prev-unit V tail at kc0-1, norm(prev) at kc2, own V at
            #      lag 3; 3-chunk V tail spills forward.
            units = [(0, 0), (1, 0), (0, 1), (1, 1)]
            o2_u, exs_u = [], []
            for ui, (qh, pr) in enumerate(units):
                o2 = [psp.tile([65, 1024], F32, tag="ps", name="o2")
                      for _ in range(2)]
                exs = []
                o2_u.append(o2)
                exs_u.append(exs)
                ppr = units[ui - 1][1] if ui else 0
                pqh = units[ui - 1][0] if ui else 0
                for kc in range(KC):
                    emit_scores(qh, pr, kc, exs)
                    if ui == 0:
                        # kproj-th1 / qproj-th1 quarter-waves timed to their
                        # DMA arrivals; vproj + lag-10 V fill the rest.
                        if kc == 2:
                            emit_projq(wk_sb, xk_sb, kT_sb, bk_sb, 1, 0, "v")
                        elif kc == 4:
                            emit_projq(wk_sb, xk_sb, kT_sb, bk_sb, 1, 1, "v")
                        elif kc == 7:
                            emit_projq(wq_sb, xq_sb, qT_sb, bq_sb, 1, 0, "v")
                        elif kc == 9:
                            emit_projq(wq_sb, xq_sb, qT_sb, bq_sb, 1, 1, "v")
                        elif kc in (5, 6, 8):
                            emit_vproj({5: 0, 6: 1, 8: 2}[kc])
                        if kc >= 10:
                            emit_vproj(kc - 7)
                            emit_v(0, kc - 10, o2, exs)
                    elif ui == 1:
                        if kc <= 2:
                            emit_vproj(9 + 2 * kc)
                            emit_vproj(10 + 2 * kc)
                        elif kc == 3:
                            emit_vproj(15)
                            emit_v(0, 6, o2_u[0], exs_u[0])
                        elif kc <= 7:
                            emit_v(0, 2 * kc - 1, o2_u[0], exs_u[0])
                            emit_v(0, 2 * kc, o2_u[0], exs_u[0])
                        elif kc == 8:
                            emit_v(0, 15, o2_u[0], exs_u[0])
                            emit_norm(0, 0, o2_u[0])
                        else:
                            # kc 9..15 -> V(u1, 0..9) at 1.5/kc
                            c0 = (3 * (kc - 9)) // 2
                            c1 = (3 * (kc - 8)) // 2
                            for c in range(c0, c1):
                                emit_v(pr, c, o2, exs)
                    else:
                        if kc <= 2 and ui == 2:
                            emit_v(ppr, 8 + 2 * kc + 2, o2_u[1], exs_u[1])
                            emit_v(ppr, 8 + 2 * kc + 3, o2_u[1], exs_u[1])
                        elif kc <= 1 and ui == 3:
                            emit_v(ppr, 12 + 2 * kc, o2_u[2], exs_u[2])
                            emit_v(ppr, 13 + 2 * kc, o2_u[2], exs_u[2])
                        elif kc == 3 and ui == 2 or kc == 2 and ui == 3:
                            emit_norm(pqh, ppr, o2_u[ui - 1])
                        if ui == 2 and kc >= 4:
                            emit_v(pr, kc - 4, o2, exs)
                        elif ui == 3 and kc >= 3:
                            emit_v(pr, kc - 3, o2, exs)

            # tail: V(u3) x3 -> outproj(0) brackets norm(u3) -> outproj(1)
            for kc in range(KC - 3, KC):
                emit_v(pr, kc, o2, exs)
            for tr in range(4):
                emit_outproj(0, tr, ceng="sv"[tr % 2])
            emit_norm(1, 1, o2)
            for tr in range(4, 8):
                emit_outproj(0, tr, ceng="sv"[tr % 2])
            for tr in range(8):
                emit_outproj(1, tr, ceng="sv"[tr % 2])

            xkp_cm.__exit__(None, None, None)
            xqp_cm.__exit__(None, None, None)

            for _p in reversed(bc_pools):
                _p.__exit__(None, None, None)
            xvp_cm.__exit__(None, None, None)

    nc.compile()
    return nc


_CACHE = {}


def _get_program(with_bv: bool, with_qkb: bool):
    key = (with_bv, with_qkb)
    if key not in _CACHE:
        _CACHE[key] = build_program(with_bv, with_qkb)
    return _CACHE[key]


def make_in_maps(query, key, value, mask, Wq, bq, Wk, bk, Wv, bv, Wo, bo):
    bf = ml_dtypes.bfloat16
    f8 = ml_dtypes.float8_e4m3
    # transposed activations are shared by the 4 cores of each batch
    xt = {}
    for nm, x, dt in (("xq", query, bf), ("xk", key, bf)):
        for b in range(B):
            xt[nm, b] = np.ascontiguousarray(
                x[b].T.reshape(DCH, 128, T)).astype(dt)
    for b in range(B):
        # [KC, 128, DCH, 128]: token-chunk major, partition = D-row within
        # chunk, 2KB contiguous per-partition lines for full DMA speed
        xt["xv", b] = np.ascontiguousarray(
            value[b].reshape(KC, 128, DCH, 128).transpose(0, 3, 2, 1)
        ).astype(bf)
    in_maps = []
    for c in range(NCORES):
        b, g = c // 4, c % 4
        cols = slice(GC * g, GC * (g + 1))
        m = {}
        for nm in ("xq", "xk", "xv"):
            m[nm] = xt[nm, b]
        for nm, W, dt in (("wq", Wq, bf), ("wk", Wk, bf), ("wv", Wv, bf)):
            m[nm] = np.ascontiguousarray(
                W[cols, :].T.reshape(DCH, 128, GC).transpose(1, 0, 2)
            ).astype(dt)
        m["wo"] = np.ascontiguousarray(
            Wo[:, cols].T.reshape(2, 128, D).transpose(1, 0, 2)).astype(bf)
        mb = np.where(mask[b, 0] != 0, 0.0, MASK_NEG).astype(np.float32)
        m["maskb"] = np.ascontiguousarray(mb.reshape(KC, 128).T)
        m["bq"] = np.ascontiguousarray(
            bq[cols].reshape(2, 128).T.astype(np.float32))
        m["bk"] = np.ascontiguousarray(
            bk[cols].reshape(2, 128).T.astype(np.float32))
        m["bv"] = np.ascontiguousarray(
            bv[cols].reshape(GH, 64).T.astype(np.float32))
        in_maps.append(m)
    return in_maps


def kernel(query, key, value, mask, Wq, bq, Wk, bk, Wv, bv, Wo, bo,
           _trace=False):
    query, key, value = (np.asarray(a, np.float32) for a in (query, key, value))
    mask = np.asarray(mask)
    with_bv = bool(np.any(np.asarray(bv)))
    with_qkb = bool(np.any(np.asarray(bq))) or bool(np.any(np.asarray(bk)))
    nc = _get_program(with_bv, with_qkb)
    in_maps = make_in_maps(query, key, value, mask, Wq, bq, Wk, bk, Wv, bv,
                           Wo, bo)
    res = run_bass_kernel_spmd(nc, in_maps, list(range(NCORES)), trace=_trace)
    out = np.zeros((B, T, D), np.float32)
    for c in range(NCORES):
        out[c // 4] += res.results[c]["out"].astype(np.float32)
    out += np.asarray(bo, np.float32)[None, None, :]
    if _trace:
        kernel.last_exec_time_ns = res.exec_time_ns
        kernel.last_results = res
    return out


# revision 24
# speedup vs baseline: 1.0156x; 1.0130x over previous
"""MultiHeadedAttention Trainium2 kernel (v3.1 — exp-stream-centred pipeline).

Problem: B=2, T=2048, D=1024, H=16 heads (DK=64), fp32 in/out, padding mask
on keys. out = softmax(mask(QWq (KWk)^T / 8)) @ (VWv) @ Wo^T + biases.

Sharding (8 cores): core c -> batch b = c//4, head group g = c%4 (4 heads,
256 projection columns). Each core computes its heads' attention and a
partial output projection; host sums the 4 partials per batch (+ bo).

The ScalarE exp stream (128 x ~1.0us ACTIVATEs, cadence (N+172)/1.2) is the
hard floor (~128us). Everything else is arranged around keeping it saturated:
  - Phase A: all four (m, th) k-projection accumulators run interleaved,
    k-outer, paced by the xk chunk DMAs (full [128,2048] chunks -> 4KB DMA
    lines, the fastest upload shape); then the same for q. Evacuation copies
    that gate the first scores go to the still-idle ScalarE.
  - x chunks live in 3-slot ring buffers (consumed in arrival order).
  - vproj rides unit 0 (chunk kc-2 at step kc); V(u0) runs at lag 8 and
    spills its tail into unit 1, which catches up with 2-per-step V; units
    2/3 run the steady lag-2 schedule so the tail V is only 2 chunks.
  - PSUM is exactly 4 rotating 2-bank slots: 2 score tiles in flight + the
    2 V accumulators (o2).  Riders needing a 5th slot stall the exp stream
    briefly; outproj therefore runs only in the tail.
  - V accumulators are evacuated to SBUF immediately after stop (frees the
    PSUM slots); normalize runs SBUF-side (recip + gpsimd bcast + bf16 mul),
    hh1 chain first (its extra DMA partition-shift overlaps hh0's DVE work).
  - tail: V(u3) x2 -> outproj(0) brackets norm(u3) -> outproj(1).
"""

import numpy as np
import ml_dtypes

import concourse.bass as bass
import concourse.bacc as bacc
import concourse.tile as tile
from concourse import mybir
from concourse.bass_utils import run_bass_kernel_spmd

B, T, D, H = 2, 2048, 1024, 16
DK = D // H  # 64
GH = 4       # heads per core
GC = GH * DK  # 256 proj columns per core
NCORES = 8
KC = T // 128   # 16 key chunks
DCH = D // 128  # 8 contraction chunks
F32 = mybir.dt.float32
BF16 = mybir.dt.bfloat16

MASK_NEG = -30000.0


def build_program(with_bv: bool, with_qkb: bool):
    nc = bacc.Bacc("TRN2")

    # ---- DRAM parameters (per-core shapes) ----
    xq_d = nc.declare_dram_parameter("xq", [DCH, 128, T], BF16, isOutput=False)
    xk_d = nc.declare_dram_parameter("xk", [DCH, 128, T], BF16, isOutput=False)
    xv_d = nc.declare_dram_parameter("xv", [DCH, 128, T], BF16, isOutput=False)
    wq_d = nc.declare_dram_parameter("wq", [128, DCH, GC], BF16, isOutput=False)
    wk_d = nc.declare_dram_parameter("wk", [128, DCH, GC], BF16, isOutput=False)
    wv_d = nc.declare_dram_parameter("wv", [128, DCH, GC], BF16, isOutput=False)
    wo_d = nc.declare_dram_parameter("wo", [128, 2, D], BF16, isOutput=False)
    mask_d = nc.declare_dram_parameter("maskb", [128, KC], F32, isOutput=False)
    bq_d = nc.declare_dram_parameter("bq", [128, 2], F32, isOutput=False)
    bk_d = nc.declare_dram_parameter("bk", [128, 2], F32, isOutput=False)
    bv_d = nc.declare_dram_parameter("bv", [64, GH], F32, isOutput=False)
    out_d = nc.declare_dram_parameter("out", [T, D], BF16, isOutput=True)

    with tile.TileContext(nc) as tc:
        with (
            tc.tile_pool(name="persist", bufs=1) as pp,
            tc.tile_pool(name="psum", bufs=4, space="PSUM") as psp,
        ):
            # persistent sbuf tensors
            wv_sb = pp.tile([128, DCH, GC], BF16, tag="wv")
            wo_sb = pp.tile([128, 2, D], BF16, tag="wo")
            mask_sb = pp.tile([128, KC], F32, tag="mask")
            bq_sb = pp.tile([128, 2], F32, tag="bq")
            bk_sb = pp.tile([128, 2], F32, tag="bk")
            bv_sb = pp.tile([64, GH], F32, tag="bv")
            qT_sb = pp.tile([128, 2, T], BF16, tag="qT")
            kT_sb = pp.tile([128, 2, T], BF16, tag="kT")
            v_sb = pp.tile([128, KC, GH, 66], BF16, tag="v")
            xh_sb = [pp.tile([128, 2, 1024], BF16, tag=f"xh{q}", name=f"xh{q}")
                     for q in (0, 1)]
            # prime the ScalarE activation table (exp) before any real work
            scrap_i = pp.tile([1, 16], F32, tag="scrap_i")
            scrap_o = pp.tile([1, 16], BF16, tag="scrap_o")
            nc.vector.memset(scrap_i[:], 0.0)
            nc.scalar.activation(scrap_o[:], scrap_i[:],
                                 mybir.ActivationFunctionType.Exp)
            nc.vector.memset(v_sb[:, :, :, 64:65], 1.0)

            xvp_cm = tc.tile_pool(name="xv", bufs=1)
            xvp = xvp_cm.__enter__()
            xv_sb = [xvp.tile([128, T], BF16, tag=f"xv{k}", name=f"xv{k}")
                     for k in range(DCH)]

            bc_pools = (
                tc.tile_pool(name="expp", bufs=24),
                tc.tile_pool(name="outp", bufs=4),
                tc.tile_pool(name="normp", bufs=2),
            )
            exp_pool = bc_pools[0].__enter__()
            out_pool = bc_pools[1].__enter__()
            norm_pool = bc_pools[2].__enter__()

            xqp_cm = tc.tile_pool(name="xqp", bufs=1)
            xqp = xqp_cm.__enter__()
            wq_sb = xqp.tile([128, DCH, GC], BF16, tag="wq", name="wq_sb")
            xq_sb = [xqp.tile([128, T], BF16, tag="xq", bufs=3, name=f"xq{k}")
                     for k in range(DCH)]
            xkp_cm = tc.tile_pool(name="xkp", bufs=1)
            xkp = xkp_cm.__enter__()
            wk_sb = xkp.tile([128, DCH, GC], BF16, tag="wk", name="wk_sb")
            xk_sb = [xkp.tile([128, T], BF16, tag="xk", bufs=3, name=f"xk{k}")
                     for k in range(DCH)]

            # ---- DMAs in priority order: k -> q -> v ----
            nc.sync.dma_start(out=wk_sb[:], in_=wk_d[:])
            nc.sync.dma_start(out=bk_sb[:], in_=bk_d[:])
            for k in range(DCH):
                nc.sync.dma_start(out=xk_sb[k][:], in_=xk_d[k])
            nc.sync.dma_start(out=wq_sb[:], in_=wq_d[:])
            nc.sync.dma_start(out=bq_sb[:], in_=bq_d[:])
            nc.sync.dma_start(out=mask_sb[:], in_=mask_d[:])
            for k in range(DCH):
                nc.sync.dma_start(out=xq_sb[k][:], in_=xq_d[k])
            nc.sync.dma_start(out=wv_sb[:], in_=wv_d[:])
            for k in range(DCH):
                nc.sync.dma_start(out=xv_sb[k][:], in_=xv_d[k])
            nc.sync.dma_start(out=bv_sb[:], in_=bv_d[:])
            nc.sync.dma_start(out=wo_sb[:], in_=wo_d[:])

            def emit_proj4(w_sb, x_sb, dst, b_sb, order):
                """All four (m, th) quarters of a projection, k-outer so the
                accumulators advance in lockstep with the x-chunk DMAs.
                `order`: list of (m, th, engine) for the evacuation copies."""
                pst = {}
                for m in (0, 1):
                    for th in (0, 1):
                        pst[(m, th)] = psp.tile([128, 1024], F32, tag="ps",
                                                name=f"pst{m}{th}")
                for k in range(DCH):
                    for m in (0, 1):
                        for th in (0, 1):
                            for n in (0, 1):
                                nc.tensor.matmul(
                                    pst[(m, th)][:, n * 512:(n + 1) * 512],
                                    w_sb[:, k, m * 128:(m + 1) * 128],
                                    x_sb[k][:, th * 1024 + n * 512:
                                            th * 1024 + (n + 1) * 512],
                                    start=(k == 0), stop=(k == DCH - 1),
                                    skip_group_check=True,
                                )
                for m, th, eng in order:
                    sl = dst[:, m, th * 1024:(th + 1) * 1024]
                    if with_qkb:
                        nc.vector.tensor_scalar_add(sl, pst[(m, th)][:],
                                                    b_sb[:, m:m + 1])
                    elif eng == "s":
                        nc.scalar.copy(sl, pst[(m, th)][:])
                    else:
                        nc.vector.tensor_copy(sl, pst[(m, th)][:])

            def emit_vproj(tcn):
                ps = psp.tile([128, GH, 64], F32, tag="ps", name="vps")
                for k in range(DCH):
                    nc.tensor.matmul(
                        ps[:],
                        xv_sb[k][:, tcn * 128:(tcn + 1) * 128],
                        wv_sb[:, k, :],
                        start=(k == 0), stop=(k == DCH - 1),
                        skip_group_check=True,
                    )
                nc.vector.tensor_copy(v_sb[:, tcn, :, 0:64], ps[:])

            # ---- Phase A: k then q projections, all 4 quarters DMA-paced ----
            emit_proj4(wk_sb, xk_sb, kT_sb, bk_sb,
                       [(0, 0, "s"), (1, 0, "v"), (0, 1, "s"), (1, 1, "v")])
            xkp_cm.__exit__(None, None, None)
            emit_proj4(wq_sb, xq_sb, qT_sb, bq_sb,
                       [(0, 0, "s"), (1, 0, "v"), (0, 1, "s"), (1, 1, "v")])
            xqp_cm.__exit__(None, None, None)

            def emit_scores(qh, pr, kc, exs):
                q0 = qh * 1024
                se = [psp.tile([128, 1024], F32, tag="ps", name="se")
                      for _ in range(2)]
                for n in range(2):
                    for hh in range(2):
                        pb = 64 * hh
                        nc.tensor.matmul(
                            se[hh][:, n * 512:(n + 1) * 512],
                            kT_sb[pb:pb + 64, pr, kc * 128:(kc + 1) * 128],
                            qT_sb[pb:pb + 64, pr,
                                  q0 + n * 512:q0 + (n + 1) * 512],
                            start=True, stop=True,
                        )
                ex = [exp_pool.tile([128, 1024], BF16, tag="ex", name="ex")
                      for _ in range(2)]
                for hh in range(2):
                    nc.scalar.activation(
                        ex[hh][:], se[hh][:],
                        mybir.ActivationFunctionType.Exp,
                        bias=mask_sb[:, kc:kc + 1],
                        scale=float(DK) ** -0.5,
                    )
                exs.append(ex)

            def emit_v(pr, kc, o2, exs):
                for hh in range(2):
                    h = 2 * pr + hh
                    for n in range(2):
                        nc.tensor.matmul(
                            o2[hh][:, n * 512:(n + 1) * 512],
                            v_sb[:, kc, h, 0:65],
                            exs[kc][hh][:, n * 512:(n + 1) * 512],
                            start=(kc == 0), stop=(kc == KC - 1),
                            skip_group_check=True,
                        )

            def emit_norm(qh, pr, o2):
                """Evacuate o2 fast (frees PSUM), then normalize SBUF-side.
                hh1 first: its DMA partition-shift overlaps hh0's DVE work."""
                rr, xr, rc, rrb, rb = [None, None], [None, None], \
                    [None, None], [None, None], [None, None]
                for hh in (1, 0):
                    rr[hh] = norm_pool.tile([1, 1024], F32, tag="rr",
                                            name="rr")
                    nc.vector.tensor_copy(rr[hh][:], o2[hh][64:65, :])
                    xr[hh] = norm_pool.tile([64, 1024], BF16, tag="xr",
                                            name="xr")
                    nc.vector.tensor_copy(xr[hh][:], o2[hh][0:64, :])
                for hh in (1, 0):
                    rc[hh] = norm_pool.tile([1, 1024], F32, tag="rc",
                                            name="rc")
                    nc.vector.reciprocal_approx_fast(rc[hh][:], rr[hh][:])
                    rrb[hh] = norm_pool.tile([1, 1024], BF16, tag="rrb",
                                             bufs=1, name="rrb")
                    nc.vector.tensor_copy(rrb[hh][:], rc[hh][:])
                    rb[hh] = norm_pool.tile([64, 1024], BF16, tag="rb",
                                            name="rb")
                    nc.gpsimd.partition_broadcast(rb[hh][:], rrb[hh][:])
                for hh in (1, 0):
                    if hh == 0:
                        nc.vector.tensor_mul(
                            xh_sb[qh][0:64, pr, :], xr[hh][:], rb[hh][:])
                        if with_bv:
                            nc.vector.tensor_scalar_add(
                                xh_sb[qh][0:64, pr, :],
                                xh_sb[qh][0:64, pr, :],
                                bv_sb[:, 2 * pr:2 * pr + 1])
                    else:
                        tmp = norm_pool.tile([64, 1024], BF16, tag="tmp",
                                             name="tmp")
                        nc.vector.tensor_mul(tmp[:], xr[hh][:], rb[hh][:])
                        if with_bv:
                            nc.vector.tensor_scalar_add(
                                tmp[:], tmp[:],
                                bv_sb[:, 2 * pr + 1:2 * pr + 2])
                        nc.sync.dma_start(
                            out=xh_sb[qh][64:128, pr, :], in_=tmp[:])

            def emit_outproj(qh, tr, ceng="v"):
                tcn = qh * 8 + tr
                po = psp.tile([128, 1024], F32, tag="ps", name="po")
                for m in range(2):
                    for n in range(2):
                        nc.tensor.matmul(
                            po[:, n * 512:(n + 1) * 512],
                            xh_sb[qh][:, m, tr * 128:(tr + 1) * 128],
                            wo_sb[:, m, n * 512:(n + 1) * 512],
                            start=(m == 0), stop=(m == 1),
                            skip_group_check=True,
                        )
                ot = out_pool.tile([128, 1024], BF16, tag="ot")
                if ceng == "s":
                    nc.scalar.copy(ot[:], po[:])
                else:
                    nc.vector.tensor_copy(ot[:], po[:])
                nc.sync.dma_start(
                    out=out_d[tcn * 128:(tcn + 1) * 128, :], in_=ot[:])

            # units ordered (0,0),(1,0),(0,1),(1,1)
            units = [(0, 0), (1, 0), (0, 1), (1, 1)]
            o2_u, exs_u = [], []
            for ui, (qh, pr) in enumerate(units):
                o2 = [psp.tile([65, 1024], F32, tag="ps", name="o2")
                      for _ in range(2)]
                exs = []
                o2_u.append(o2)
                exs_u.append(exs)
                for kc in range(KC):
                    emit_scores(qh, pr, kc, exs)
                    if ui == 0:
                        if 2 <= kc:
                            emit_vproj(kc - 2)
                        if kc >= 8:
                            emit_v(0, kc - 8, o2, exs)
                    elif ui == 1:
                        if kc == 0:
                            emit_vproj(14)
                            emit_v(0, 8, o2_u[0], exs_u[0])
                        elif kc == 1:
                            emit_vproj(15)
                            emit_v(0, 9, o2_u[0], exs_u[0])
                        elif kc <= 4:
                            emit_v(0, 2 * kc + 6, o2_u[0], exs_u[0])
                            emit_v(0, 2 * kc + 7, o2_u[0], exs_u[0])
                        elif kc == 5:
                            emit_norm(0, 0, o2_u[0])  # frees o2(u0)
                        elif kc <= 10:
                            emit_v(pr, kc - 6, o2, exs)
                        else:
                            emit_v(pr, 2 * kc - 17, o2, exs)
                            emit_v(pr, 2 * kc - 16, o2, exs)
                    else:
                        if kc >= 2:
                            emit_v(pr, kc - 2, o2, exs)
                if ui == 1:
                    emit_v(pr, KC - 1, o2, exs)
                elif ui >= 2:
                    for kc in range(KC - 2, KC):
                        emit_v(pr, kc, o2, exs)
                if ui == 1 or ui == 2:
                    emit_norm(qh, pr, o2)

            # tail: outproj(0) brackets norm(u3) so the PE never idles long
            # enough to re-throttle; then outproj(1).
            for tr in range(4):
                emit_outproj(0, tr, ceng="sv"[tr % 2])
            emit_norm(1, 1, o2)
            for tr in range(4, 8):
                emit_outproj(0, tr, ceng="sv"[tr % 2])
            for tr in range(8):
                emit_outproj(1, tr, ceng="sv"[tr % 2])

            for _p in reversed(bc_pools):
                _p.__exit__(None, None, None)
            xvp_cm.__exit__(None, None, None)

    nc.compile()
    return nc


_CACHE = {}


def _get_program(with_bv: bool, with_qkb: bool):
    key = (with_bv, with_qkb)
    if key not in _CACHE:
        _CACHE[key] = build_program(with_bv, with_qkb)
    return _CACHE[key]


def make_in_maps(query, key, value, mask, Wq, bq, Wk, bk, Wv, bv, Wo, bo):
    bf = ml_dtypes.bfloat16
    # transposed bf16 activations are shared by the 4 cores of each batch
    xt = {}
    for nm, x in (("xq", query), ("xk", key), ("xv", value)):
        for b in range(B):
            xt[nm, b] = np.ascontiguousarray(
                x[b].T.reshape(DCH, 128, T)).astype(bf)
    in_maps = []
    for c in range(NCORES):
        b, g = c // 4, c % 4
        cols = slice(GC * g, GC * (g + 1))
        m = {}
        for nm in ("xq", "xk", "xv"):
            m[nm] = xt[nm, b]
        for nm, W in (("wq", Wq), ("wk", Wk), ("wv", Wv)):
            m[nm] = np.ascontiguousarray(
                W[cols, :].T.reshape(DCH, 128, GC).transpose(1, 0, 2)
            ).astype(bf)
        m["wo"] = np.ascontiguousarray(
            Wo[:, cols].T.reshape(2, 128, D).transpose(1, 0, 2)).astype(bf)
        mb = np.where(mask[b, 0] != 0, 0.0, MASK_NEG).astype(np.float32)
        m["maskb"] = np.ascontiguousarray(mb.reshape(KC, 128).T)
        m["bq"] = np.ascontiguousarray(
            bq[cols].reshape(2, 128).T.astype(np.float32))
        m["bk"] = np.ascontiguousarray(
            bk[cols].reshape(2, 128).T.astype(np.float32))
        m["bv"] = np.ascontiguousarray(
            bv[cols].reshape(GH, 64).T.astype(np.float32))
        in_maps.append(m)
    return in_maps


def kernel(query, key, value, mask, Wq, bq, Wk, bk, Wv, bv, Wo, bo,
           _trace=False):
    query, key, value = (np.asarray(a, np.float32) for a in (query, key, value))
    mask = np.asarray(mask)
    with_bv = bool(np.any(np.asarray(bv)))
    with_qkb = bool(np.any(np.asarray(bq))) or bool(np.any(np.asarray(bk)))
    nc = _get_program(with_bv, with_qkb)
    in_maps = make_in_maps(query, key, value, mask, Wq, bq, Wk, bk, Wv, bv,
                           Wo, bo)
    res = run_bass_kernel_spmd(nc, in_maps, list(range(NCORES)), trace=_trace)
    out = np.zeros((B, T, D), np.float32)
    for c in range(NCORES):
        out[c // 4] += res.results[c]["out"].astype(np.float32)
    out += np.asarray(bo, np.float32)[None, None, :]
    if _trace:
        kernel.last_exec_time_ns = res.exec_time_ns
        kernel.last_results = res
    return out
